# revision 6
# baseline (speedup 1.0000x reference)
"""Trainium2 Bass kernel for nn_Air_Model (Elman RNN cell over L=512 steps).

reference:
    ux = einsum("bln,ns->bls", x, U_w) + U_b          # [B, L, S]
    scan over l: a = relu(ux_l + a @ W_w + W_b)       # a: [B, S]
    out = a_last @ V_w + V_b                          # [B, M]

Shapes: B=4096, L=512, N=12, S=128, M=12 (fp32).

Strategy (data-parallel over batch, 8 cores, B_local=512 per core):
  - Keep the scan state transposed in SBUF: A^T [S=128 part, B_local free].
  - Per step l: PSUM accumulation  psum = U^T x_l^T (K=12) + W^T A^T (K=128),
    then relu+bias on ScalarE/VectorE (split batch halves across the two
    engines, separate PSUM banks per half to satisfy bank rules).
  - x arrives [b, l, n] (n innermost) but the U-matmul needs n on partitions:
    stream x in l-blocks, pad n 12->32, and transpose [128b, 4l x 32n] blocks
    on TensorE; copy PSUM->SBUF to give the matmul its rhs.
  - Final projection uses A^T directly as lhsT: out[b, m] = A^T.T @ V_w, with
    V_b folded in via a K=1 ones-row matmul.
"""

import numpy as np

import concourse.bass as bass
import concourse.mybir as mybir
import concourse.tile as tile
from concourse import mybir as _mybir
from concourse.bass_utils import run_bass_kernel_spmd
from concourse.masks import make_identity
from concourse.vector_clock import ScopedClock
from bass_rust import SemaphoreHandle

# ---------------------------------------------------------------------------
# Patch: this walrus build supports only ONE sync-wait per instruction, but
# Tile's kernel-tail drain accumulates one wait per outstanding semaphore.
# Split them into one drain instruction per wait.
# ---------------------------------------------------------------------------


def _drain_and_barrier_split(self, tick_clock, wait_clock):
    nc = self.nc
    probe = mybir.InstDrain(name=nc.get_next_instruction_name(), ins=[], outs=[])
    probe.engine = mybir.EngineType.SP
    wait_clock.add_sem_waits(probe, ScopedClock({None: tick_clock.global_clock}))
    waits = list(probe.sync_info.on_wait) if probe.sync_info else []
    for w in waits:
        d = nc.sync.drain()
        sem = SemaphoreHandle(num=w.id, name=w.ant_name)
        d.wait_op(sem, w.wait_value, w.wait_mode.removesuffix("-imm"))
    if not waits:
        nc.sync.drain()

    nc.all_engine_barrier()
    assert self.sems is not None
    popped = nc._tile_sem_poison_stack.pop()
    assert popped is self._sem_poison
    nc.clear_and_free_semaphores(list(self.sems.allocated().values()))
    nc.all_engine_barrier()


tile.TileContext._drain_and_barrier = _drain_and_barrier_split


def _split_multi_waits(nc):
    """Walrus here allows only one sync-wait per instruction, but Tile's
    semaphore assignment can attach several. Hoist extra waits onto fresh
    NOPs placed immediately before the instruction on the same engine."""
    import bass_rust

    SyncInfo = bass_rust.SyncInfo
    n_split = 0
    for fn in nc.m.functions:
        for blk in fn.blocks:
            insts = blk.instructions
            if not any(
                i.sync_info is not None and len(i.sync_info.on_wait) > 1
                for i in insts
            ):
                continue
            new = []
            for inst in insts:
                si = inst.sync_info
                if si is not None and len(si.on_wait) > 1:
                    waits = list(si.on_wait)
                    for w in waits[:-1]:
                        nop = mybir.InstNoOp(
                            name=nc.get_next_instruction_name(), ins=[], outs=[]
                        )
                        nop.engine = inst.engine
                        nop.sync_info = SyncInfo(on_wait=[w], on_update=[])
                        new.append(nop)
                        n_split += 1
                    inst.sync_info = SyncInfo(
                        on_wait=[waits[-1]], on_update=list(si.on_update)
                    )
                new.append(inst)
            blk.instructions = new
    return n_split


# ---------------------------------------------------------------------------

B, L, N, S, M = 4096, 512, 12, 128, 12
NCORES = 8
BL = B // NCORES        # 512 local batch
NPAD = 32               # n padded so transposed l-groups are 32-aligned
LQ = 4                  # l's per [128, 128] transpose block
TBLK = 32               # l's per streamed x block
NBLK = L // TBLK        # 16 l-blocks
NBC = BL // 128         # 4 batch chunks of 128 for x streaming
HALF = BL // 2          # 256: scan processed as two independent halves

F32 = mybir.dt.float32
AF = mybir.ActivationFunctionType
ALU = mybir.AluOpType


def _build():
    nc = bass.Bass(trn_type="TRN2")

    x_d = nc.dram_tensor("x", [BL, L, N], F32, kind="ExternalInput")
    a0_d = nc.dram_tensor("a0", [BL, S], F32, kind="ExternalInput")
    Uw_d = nc.dram_tensor("U_w", [N, S], F32, kind="ExternalInput")
    Ub_d = nc.dram_tensor("U_b", [S], F32, kind="ExternalInput")
    Ww_d = nc.dram_tensor("W_w", [S, S], F32, kind="ExternalInput")
    Wb_d = nc.dram_tensor("W_b", [S], F32, kind="ExternalInput")
    Vw_d = nc.dram_tensor("V_w", [S, M], F32, kind="ExternalInput")
    Vb_d = nc.dram_tensor("V_b", [M], F32, kind="ExternalInput")
    out_d = nc.dram_tensor("out", [BL, M], F32, kind="ExternalOutput")

    with tile.TileContext(nc) as tc:
        with (
            tc.tile_pool(name="singles", bufs=1) as singles,
            tc.tile_pool(name="ps_scan", bufs=6, space="PSUM") as ps_scan,
            tc.tile_pool(name="ps_xt", bufs=2, space="PSUM") as ps_xt,
        ):
            # ---- parameters ------------------------------------------------
            w_sb = singles.tile([S, S], F32, tag="w")          # W_w as lhsT
            nc.sync.dma_start(out=w_sb, in_=Ww_d[:, :])

            u_sb = singles.tile([128, S], F32, tag="u")        # U replicated x4
            for g in range(4):
                nc.sync.dma_start(out=u_sb[32 * g : 32 * g + N, :], in_=Uw_d[:, :])

            ub_sb = singles.tile([S, 1], F32, tag="ub")
            nc.sync.dma_start(out=ub_sb, in_=Ub_d[:].rearrange("(s o) -> s o", o=1))
            wb_sb = singles.tile([S, 1], F32, tag="wb")
            nc.sync.dma_start(out=wb_sb, in_=Wb_d[:].rearrange("(s o) -> s o", o=1))
            bias_sb = singles.tile([S, 1], F32, tag="bias")
            nc.vector.tensor_tensor(
                out=bias_sb, in0=ub_sb, in1=wb_sb, op=ALU.add
            )

            v_sb = singles.tile([S, M], F32, tag="v")
            nc.sync.dma_start(out=v_sb, in_=Vw_d[:, :])
            vb_row = singles.tile([1, M], F32, tag="vb")
            nc.sync.dma_start(out=vb_row, in_=Vb_d[:].rearrange("(o m) -> o m", o=1))
            ones_row = singles.tile([1, 128], F32, tag="ones")
            nc.vector.memset(ones_row, 1.0)

            ident = singles.tile([128, 128], F32, tag="ident")
            make_identity(nc, ident)

            # ---- scan state A^T [S, BL], ping-pong -------------------------
            a_t = [singles.tile([S, BL], F32, tag=f"a{i}", name=f"a{i}") for i in range(2)]

            # load a0 -> A^T via 4 TensorE transposes
            for cb in range(NBC):
                a0_sb = singles.tile([128, S], F32, tag=f"a0in{cb}")
                nc.sync.dma_start(
                    out=a0_sb, in_=a0_d[cb * 128 : (cb + 1) * 128, :]
                )
                pt = ps_xt.tile([128, 512], F32, tag="xtp")
                nc.tensor.transpose(pt[:, 0:128], a0_sb, ident)
                nc.scalar.copy(
                    out=a_t[0][:, cb * 128 : (cb + 1) * 128], in_=pt[:, 0:128]
                )

            # ---- x streaming buffers --------------------------------------
            # xp[j][cb]: [128b, TBLK*NPAD] padded input block (j = block parity)
            # xt[j][q] : [128 (4l x 32n), BL] transposed block, q = quad in blk
            xp = [
                [singles.tile([128, TBLK * NPAD], F32, tag=f"xp{j}_{cb}",
                              name=f"xp{j}_{cb}")
                 for cb in range(NBC)]
                for j in range(2)
            ]
            xt = [
                [singles.tile([128, BL], F32, tag=f"xt{j}_{q}",
                              name=f"xt{j}_{q}")
                 for q in range(TBLK // LQ)]
                for j in range(2)
            ]
            # zero-fill once so pad columns stay finite & initialized
            for j in range(2):
                for cb in range(NBC):
                    nc.gpsimd.memset(xp[j][cb], 0.0)

            # ---- main loop -------------------------------------------------
            for jb in range(NBLK):
                j = jb % 2
                # stream x block: [128, TBLK, 12] -> padded [128, TBLK, 32]
                for cb in range(NBC):
                    dst = xp[j][cb].rearrange("p (l n) -> p l n", n=NPAD)[:, :, 0:N]
                    nc.sync.dma_start(
                        out=dst,
                        in_=x_d[
                            cb * 128 : (cb + 1) * 128,
                            jb * TBLK : (jb + 1) * TBLK,
                            :,
                        ],
                    )
                # transpose quads: 4 b-chunks -> one [128, BL] xt tile
                for q in range(TBLK // LQ):
                    pt = ps_xt.tile([128, 512], F32, tag="xtp")
                    for cb in range(NBC):
                        nc.tensor.transpose(
                            pt[:, cb * 128 : (cb + 1) * 128],
                            xp[j][cb][:, q * 128 : (q + 1) * 128],
                            ident,
                        )
                    if q % 4 == 3:
                        nc.vector.tensor_copy(xt[j][q], pt)
                    else:
                        nc.scalar.copy(out=xt[j][q], in_=pt)

                # scan steps of this block
                for lt in range(TBLK):
                    l = jb * TBLK + lt
                    q, g = lt // LQ, lt % LQ
                    a_prev = a_t[l % 2]
                    a_new = a_t[(l + 1) % 2]
                    for h in range(2):
                        cols = slice(h * HALF, (h + 1) * HALF)
                        ps = ps_scan.tile([128, HALF], F32, tag="scan")
                        nc.tensor.matmul(
                            ps,
                            u_sb[32 * g : 32 * g + N, :],
                            xt[j][q][32 * g : 32 * g + N, cols],
                            start=True,
                            stop=False,
                            tile_position=(32 * g, 0),
                        )
                        nc.tensor.matmul(
                            ps, w_sb, a_prev[:, cols], start=False, stop=True
                        )
                        if h == 0:
                            nc.scalar.activation(
                                a_new[:, cols], ps, AF.Relu, bias=bias_sb, scale=1.0
                            )
                        else:
                            nc.vector.tensor_scalar(
                                out=a_new[:, cols],
                                in0=ps,
                                scalar1=bias_sb,
                                scalar2=0.0,
                                op0=ALU.add,
                                op1=ALU.max,
                            )

            # ---- output: out[b, m] = A^T.T @ V_w + V_b ---------------------
            a_last = a_t[L % 2]
            for cb in range(NBC):
                po = ps_xt.tile([128, 512], F32, tag="xtp")
                nc.tensor.matmul(
                    po[:, 0:M], ones_row, vb_row, start=True, stop=False
                )
                nc.tensor.matmul(
                    po[:, 0:M],
                    a_last[:, cb * 128 : (cb + 1) * 128],
                    v_sb,
                    start=False,
                    stop=True,
                )
                o_sb = singles.tile([128, M], F32, tag=f"osb{cb}")
                nc.scalar.copy(out=o_sb, in_=po[:, 0:M])
                nc.sync.dma_start(
                    out=out_d[cb * 128 : (cb + 1) * 128, :], in_=o_sb
                )

    _split_multi_waits(nc)
    return nc


_CACHED_NC = None


def _get_nc():
    global _CACHED_NC
    if _CACHED_NC is None:
        _CACHED_NC = _build()
    return _CACHED_NC


def kernel(**inputs):
    x = np.ascontiguousarray(np.asarray(inputs["x"], dtype=np.float32))
    a0 = np.ascontiguousarray(np.asarray(inputs["a0"], dtype=np.float32))
    params = {
        k: np.ascontiguousarray(np.asarray(inputs[k], dtype=np.float32))
        for k in ("U_w", "U_b", "W_w", "W_b", "V_w", "V_b")
    }

    nc = _get_nc()
    in_maps = []
    for i in range(NCORES):
        m = {
            "x": x[i * BL : (i + 1) * BL],
            "a0": a0[i * BL : (i + 1) * BL],
        }
        m.update(params)
        in_maps.append(m)

    res = run_bass_kernel_spmd(nc, in_maps, core_ids=list(range(NCORES)))
    out = np.concatenate([res.results[i]["out"] for i in range(NCORES)], axis=0)
    return out.astype(np.float32)


# revision 11
# speedup vs baseline: 2.1808x; 2.1808x over previous
"""Trainium2 Bass kernel for nn_Air_Model (Elman RNN cell over L=512 steps).

reference:
    ux = einsum("bln,ns->bls", x, U_w) + U_b          # [B, L, S]
    scan over l: a = relu(ux_l + a @ W_w + W_b)       # a: [B, S]
    out = a_last @ V_w + V_b                          # [B, M]

Shapes: B=4096, L=512, N=12, S=128, M=12 (fp32).

Strategy (data-parallel over batch, 8 cores, B_local=512 per core):
  - Keep the scan state transposed in SBUF: A^T [S=128 part, B_local free].
  - Per step l: PSUM accumulation  psum = U^T x_l^T (K=12) + W^T A^T (K=128),
    then relu+bias on ScalarE/VectorE (split batch halves across the two
    engines, separate PSUM banks per half to satisfy bank rules).
  - x arrives [b, l, n] (n innermost) but the U-matmul needs n on partitions:
    stream x in l-blocks, pad n 12->32, and transpose [128b, 4l x 32n] blocks
    on TensorE; copy PSUM->SBUF to give the matmul its rhs.
  - Final projection uses A^T directly as lhsT: out[b, m] = A^T.T @ V_w, with
    V_b folded in via a K=1 ones-row matmul.
"""

import numpy as np

import concourse.bass as bass
import concourse.mybir as mybir
import concourse.tile as tile
from concourse import mybir as _mybir
from concourse.bass_utils import run_bass_kernel_spmd
from concourse.masks import make_identity
from concourse.vector_clock import ScopedClock
from bass_rust import SemaphoreHandle

# ---------------------------------------------------------------------------
# Patch: this walrus build supports only ONE sync-wait per instruction, but
# Tile's kernel-tail drain accumulates one wait per outstanding semaphore.
# Split them into one drain instruction per wait.
# ---------------------------------------------------------------------------


def _drain_and_barrier_split(self, tick_clock, wait_clock):
    nc = self.nc
    probe = mybir.InstDrain(name=nc.get_next_instruction_name(), ins=[], outs=[])
    probe.engine = mybir.EngineType.SP
    wait_clock.add_sem_waits(probe, ScopedClock({None: tick_clock.global_clock}))
    waits = list(probe.sync_info.on_wait) if probe.sync_info else []
    for w in waits:
        d = nc.sync.drain()
        sem = SemaphoreHandle(num=w.id, name=w.ant_name)
        d.wait_op(sem, w.wait_value, w.wait_mode.removesuffix("-imm"))
    if not waits:
        nc.sync.drain()

    nc.all_engine_barrier()
    assert self.sems is not None
    popped = nc._tile_sem_poison_stack.pop()
    assert popped is self._sem_poison
    nc.clear_and_free_semaphores(list(self.sems.allocated().values()))
    nc.all_engine_barrier()


tile.TileContext._drain_and_barrier = _drain_and_barrier_split


def _split_multi_waits(nc):
    """Walrus here allows only one sync-wait per instruction, but Tile's
    semaphore assignment can attach several. Hoist extra waits onto fresh
    NOPs placed immediately before the instruction on the same engine."""
    import bass_rust

    SyncInfo = bass_rust.SyncInfo
    n_split = 0
    for fn in nc.m.functions:
        for blk in fn.blocks:
            insts = blk.instructions
            if not any(
                i.sync_info is not None and len(i.sync_info.on_wait) > 1
                for i in insts
            ):
                continue
            new = []
            for inst in insts:
                si = inst.sync_info
                if si is not None and len(si.on_wait) > 1:
                    waits = list(si.on_wait)
                    for w in waits[:-1]:
                        nop = mybir.InstNoOp(
                            name=nc.get_next_instruction_name(), ins=[], outs=[]
                        )
                        nop.engine = inst.engine
                        nop.sync_info = SyncInfo(on_wait=[w], on_update=[])
                        new.append(nop)
                        n_split += 1
                    inst.sync_info = SyncInfo(
                        on_wait=[waits[-1]], on_update=list(si.on_update)
                    )
                new.append(inst)
            blk.instructions = new
    return n_split


# ---------------------------------------------------------------------------

B, L, N, S, M = 4096, 512, 12, 128, 12
NCORES = 8
BL = B // NCORES        # 512 local batch
NPAD = 32               # n padded so transposed l-groups are 32-aligned
LQ = 4                  # l's per [128, 128] transpose block
TBLK = 32               # l's per streamed x block
NBLK = L // TBLK        # 16 l-blocks
NBC = BL // 128         # 4 batch chunks of 128 for x streaming
HALF = BL // 2          # 256: scan processed as two independent halves

F32 = mybir.dt.float32
F32R = mybir.dt.float32r
AF = mybir.ActivationFunctionType
ALU = mybir.AluOpType


def _build():
    nc = bass.Bass(trn_type="TRN2")

    x_d = nc.dram_tensor("x", [BL, L, N], F32, kind="ExternalInput")
    a0_d = nc.dram_tensor("a0", [BL, S], F32, kind="ExternalInput")
    Uw_d = nc.dram_tensor("U_w", [N, S], F32, kind="ExternalInput")
    Ub_d = nc.dram_tensor("U_b", [S], F32, kind="ExternalInput")
    Ww_d = nc.dram_tensor("W_w", [S, S], F32, kind="ExternalInput")
    Wb_d = nc.dram_tensor("W_b", [S], F32, kind="ExternalInput")
    Vw_d = nc.dram_tensor("V_w", [S, M], F32, kind="ExternalInput")
    Vb_d = nc.dram_tensor("V_b", [M], F32, kind="ExternalInput")
    out_d = nc.dram_tensor("out", [BL, M], F32, kind="ExternalOutput")

    with tile.TileContext(nc) as tc:
        with (
            tc.tile_pool(name="singles", bufs=1) as singles,
            tc.tile_pool(name="ps_scan", bufs=6, space="PSUM") as ps_scan,
            tc.tile_pool(name="ps_xt", bufs=2, space="PSUM") as ps_xt,
        ):
            # ---- parameters ------------------------------------------------
            w_stage = singles.tile([S, S], F32, tag="wst")
            nc.sync.dma_start(out=w_stage, in_=Ww_d[:, :])
            w_sb = singles.tile([S, S], F32R, tag="w")         # W_w as lhsT
            nc.vector.tensor_copy(w_sb, w_stage)

            u_stage = singles.tile([N, S], F32, tag="ust")
            nc.sync.dma_start(out=u_stage, in_=Uw_d[:, :])
            u_sb = singles.tile([128, S], F32R, tag="u")       # U replicated x4
            for g in range(4):
                nc.vector.tensor_copy(u_sb[32 * g : 32 * g + N, :], u_stage)

            ub_sb = singles.tile([S, 1], F32, tag="ub")
            nc.sync.dma_start(out=ub_sb, in_=Ub_d[:].rearrange("(s o) -> s o", o=1))
            wb_sb = singles.tile([S, 1], F32, tag="wb")
            nc.sync.dma_start(out=wb_sb, in_=Wb_d[:].rearrange("(s o) -> s o", o=1))
            bias_sb = singles.tile([S, 1], F32, tag="bias")
            nc.vector.tensor_tensor(
                out=bias_sb, in0=ub_sb, in1=wb_sb, op=ALU.add
            )

            v_sb = singles.tile([S, M], F32, tag="v")
            nc.sync.dma_start(out=v_sb, in_=Vw_d[:, :])
            vb_row = singles.tile([1, M], F32, tag="vb")
            nc.sync.dma_start(out=vb_row, in_=Vb_d[:].rearrange("(o m) -> o m", o=1))
            ones_row = singles.tile([1, 128], F32, tag="ones")
            nc.vector.memset(ones_row, 1.0)

            ident = singles.tile([128, 128], F32, tag="ident")
            make_identity(nc, ident)

            # ---- scan state A^T [S, BL], ping-pong -------------------------
            # separate tile per (parity, half) so the two half-chains have no
            # false tile-granular dependencies between them
            a_t = [
                [
                    singles.tile([S, HALF], F32R, tag=f"a{i}_{h}", name=f"a{i}_{h}")
                    for h in range(2)
                ]
                for i in range(2)
            ]

            # load a0 -> A^T via 4 TensorE transposes
            for cb in range(NBC):
                a0_sb = singles.tile([128, S], F32, tag=f"a0in{cb}")
                nc.sync.dma_start(
                    out=a0_sb, in_=a0_d[cb * 128 : (cb + 1) * 128, :]
                )
                pt = ps_xt.tile([128, 512], F32, tag="xtp")
                nc.tensor.transpose(pt[:, 0:128], a0_sb, ident)
                h, hc = divmod(cb * 128, HALF)
                nc.scalar.copy(
                    out=a_t[0][h][:, hc : hc + 128], in_=pt[:, 0:128]
                )

            # ---- x streaming buffers --------------------------------------
            # xp[j][cb]: [128b, TBLK*NPAD] padded input block (j = block parity)
            # xt[j][q] : [128 (4l x 32n), BL] transposed block, q = quad in blk
            xp = [
                [singles.tile([128, TBLK * NPAD], F32, tag=f"xp{j}_{cb}",
                              name=f"xp{j}_{cb}")
                 for cb in range(NBC)]
                for j in range(2)
            ]
            xt = [
                [singles.tile([128, BL], F32R, tag=f"xt{j}_{q}",
                              name=f"xt{j}_{q}")
                 for q in range(TBLK // LQ)]
                for j in range(2)
            ]
            # zero-fill once so pad columns stay finite & initialized
            for j in range(2):
                for cb in range(NBC):
                    nc.gpsimd.memset(xp[j][cb], 0.0)

            # ---- main loop -------------------------------------------------
            for jb in range(NBLK):
                j = jb % 2
                # stream x block: [128, TBLK, 12] -> padded [128, TBLK, 32]
                for cb in range(NBC):
                    dst = xp[j][cb].rearrange("p (l n) -> p l n", n=NPAD)[:, :, 0:N]
                    nc.sync.dma_start(
                        out=dst,
                        in_=x_d[
                            cb * 128 : (cb + 1) * 128,
                            jb * TBLK : (jb + 1) * TBLK,
                            :,
                        ],
                    )
                # transpose quads: 4 b-chunks -> one [128, BL] xt tile
                for q in range(TBLK // LQ):
                    pt = ps_xt.tile([128, 512], F32, tag="xtp")
                    for cb in range(NBC):
                        nc.tensor.transpose(
                            pt[:, cb * 128 : (cb + 1) * 128],
                            xp[j][cb][:, q * 128 : (q + 1) * 128],
                            ident,
                        )
                    if q % 4 == 3:
                        nc.vector.tensor_copy(xt[j][q], pt)
                    else:
                        nc.scalar.copy(out=xt[j][q], in_=pt)

                # scan steps of this block
                for lt in range(TBLK):
                    l = jb * TBLK + lt
                    q, g = lt // LQ, lt % LQ
                    a_prev = a_t[l % 2]
                    a_new = a_t[(l + 1) % 2]
                    for h in range(2):
                        cols = slice(h * HALF, (h + 1) * HALF)
                        ps = ps_scan.tile([128, HALF], F32, tag="scan")
                        nc.tensor.matmul(
                            ps,
                            u_sb[32 * g : 32 * g + N, :],
                            xt[j][q][32 * g : 32 * g + N, cols],
                            start=True,
                            stop=False,
                            tile_position=(32 * g, 0),
                        )
                        nc.tensor.matmul(
                            ps, w_sb, a_prev[h], start=False, stop=True
                        )
                        if h == 0:
                            nc.scalar.activation(
                                a_new[h], ps, AF.Relu, bias=bias_sb, scale=1.0
                            )
                        else:
                            nc.vector.tensor_scalar(
                                out=a_new[h],
                                in0=ps,
                                scalar1=bias_sb,
                                scalar2=0.0,
                                op0=ALU.add,
                                op1=ALU.max,
                            )

            # ---- output: out[b, m] = A^T.T @ V_w + V_b ---------------------
            a_last = a_t[L % 2]
            for cb in range(NBC):
                h, hc = divmod(cb * 128, HALF)
                po = ps_xt.tile([128, 512], F32, tag="xtp")
                nc.tensor.matmul(
                    po[:, 0:M], ones_row, vb_row, start=True, stop=False
                )
                nc.tensor.matmul(
                    po[:, 0:M],
                    a_last[h][:, hc : hc + 128].bitcast(F32),
                    v_sb,
                    start=False,
                    stop=True,
                )
                o_sb = singles.tile([128, M], F32, tag=f"osb{cb}")
                nc.scalar.copy(out=o_sb, in_=po[:, 0:M])
                nc.sync.dma_start(
                    out=out_d[cb * 128 : (cb + 1) * 128, :], in_=o_sb
                )

    _split_multi_waits(nc)
    return nc


_CACHED_NC = None


def _get_nc():
    global _CACHED_NC
    if _CACHED_NC is None:
        _CACHED_NC = _build()
    return _CACHED_NC


def kernel(**inputs):
    x = np.ascontiguousarray(np.asarray(inputs["x"], dtype=np.float32))
    a0 = np.ascontiguousarray(np.asarray(inputs["a0"], dtype=np.float32))
    params = {
        k: np.ascontiguousarray(np.asarray(inputs[k], dtype=np.float32))
        for k in ("U_w", "U_b", "W_w", "W_b", "V_w", "V_b")
    }

    nc = _get_nc()
    in_maps = []
    for i in range(NCORES):
        m = {
            "x": x[i * BL : (i + 1) * BL],
            "a0": a0[i * BL : (i + 1) * BL],
        }
        m.update(params)
        in_maps.append(m)

    res = run_bass_kernel_spmd(nc, in_maps, core_ids=list(range(NCORES)))
    out = np.concatenate([res.results[i]["out"] for i in range(NCORES)], axis=0)
    return out.astype(np.float32)


# revision 12
# speedup vs baseline: 2.9600x; 1.3573x over previous
"""Trainium2 Bass kernel for nn_Air_Model (Elman RNN cell over L=512 steps).

reference:
    ux = einsum("bln,ns->bls", x, U_w) + U_b          # [B, L, S]
    scan over l: a = relu(ux_l + a @ W_w + W_b)       # a: [B, S]
    out = a_last @ V_w + V_b                          # [B, M]

Shapes: B=4096, L=512, N=12, S=128, M=12 (fp32).

Strategy (data-parallel over batch, 8 cores, B_local=512 per core):
  - Keep the scan state transposed in SBUF: A^T [S=128 part, B_local free].
  - Per step l: PSUM accumulation  psum = U^T x_l^T (K=12) + W^T A^T (K=128),
    then relu+bias on ScalarE/VectorE (split batch halves across the two
    engines, separate PSUM banks per half to satisfy bank rules).
  - x arrives [b, l, n] (n innermost) but the U-matmul needs n on partitions:
    stream x in l-blocks, pad n 12->32, and transpose [128b, 4l x 32n] blocks
    on TensorE; copy PSUM->SBUF to give the matmul its rhs.
  - Final projection uses A^T directly as lhsT: out[b, m] = A^T.T @ V_w, with
    V_b folded in via a K=1 ones-row matmul.
"""

import numpy as np

import concourse.bass as bass
import concourse.mybir as mybir
import concourse.tile as tile
from concourse import mybir as _mybir
from concourse.bass_utils import run_bass_kernel_spmd
from concourse.masks import make_identity
from concourse.vector_clock import ScopedClock
from bass_rust import SemaphoreHandle

# ---------------------------------------------------------------------------
# Patch: this walrus build supports only ONE sync-wait per instruction, but
# Tile's kernel-tail drain accumulates one wait per outstanding semaphore.
# Split them into one drain instruction per wait.
# ---------------------------------------------------------------------------


def _drain_and_barrier_split(self, tick_clock, wait_clock):
    nc = self.nc
    probe = mybir.InstDrain(name=nc.get_next_instruction_name(), ins=[], outs=[])
    probe.engine = mybir.EngineType.SP
    wait_clock.add_sem_waits(probe, ScopedClock({None: tick_clock.global_clock}))
    waits = list(probe.sync_info.on_wait) if probe.sync_info else []
    for w in waits:
        d = nc.sync.drain()
        sem = SemaphoreHandle(num=w.id, name=w.ant_name)
        d.wait_op(sem, w.wait_value, w.wait_mode.removesuffix("-imm"))
    if not waits:
        nc.sync.drain()

    nc.all_engine_barrier()
    assert self.sems is not None
    popped = nc._tile_sem_poison_stack.pop()
    assert popped is self._sem_poison
    nc.clear_and_free_semaphores(list(self.sems.allocated().values()))
    nc.all_engine_barrier()


tile.TileContext._drain_and_barrier = _drain_and_barrier_split


def _split_multi_waits(nc):
    """Walrus here allows only one sync-wait per instruction, but Tile's
    semaphore assignment can attach several. Hoist extra waits onto fresh
    NOPs placed immediately before the instruction on the same engine."""
    import bass_rust

    SyncInfo = bass_rust.SyncInfo
    n_split = 0
    for fn in nc.m.functions:
        for blk in fn.blocks:
            insts = blk.instructions
            if not any(
                i.sync_info is not None and len(i.sync_info.on_wait) > 1
                for i in insts
            ):
                continue
            new = []
            for inst in insts:
                si = inst.sync_info
                if si is not None and len(si.on_wait) > 1:
                    waits = list(si.on_wait)
                    for w in waits[:-1]:
                        nop = mybir.InstNoOp(
                            name=nc.get_next_instruction_name(), ins=[], outs=[]
                        )
                        nop.engine = inst.engine
                        nop.sync_info = SyncInfo(on_wait=[w], on_update=[])
                        new.append(nop)
                        n_split += 1
                    inst.sync_info = SyncInfo(
                        on_wait=[waits[-1]], on_update=list(si.on_update)
                    )
                new.append(inst)
            blk.instructions = new
    return n_split


# ---------------------------------------------------------------------------

B, L, N, S, M = 4096, 512, 12, 128, 12
NCORES = 8
BL = B // NCORES        # 512 local batch
NPAD = 32               # n padded so transposed l-groups are 32-aligned
LQ = 4                  # l's per [128, 128] transpose block
TBLK = 32               # l's per streamed x block
NBLK = L // TBLK        # 16 l-blocks
NBC = BL // 128         # 4 batch chunks of 128 for x streaming
HALF = BL // 2          # 256: scan processed as two independent halves

F32 = mybir.dt.float32
F32R = mybir.dt.float32r
BF16 = mybir.dt.bfloat16
AF = mybir.ActivationFunctionType
ALU = mybir.AluOpType


def _build():
    nc = bass.Bass(trn_type="TRN2")

    x_d = nc.dram_tensor("x", [BL, L, N], F32, kind="ExternalInput")
    a0_d = nc.dram_tensor("a0", [BL, S], F32, kind="ExternalInput")
    Uw_d = nc.dram_tensor("U_w", [N, S], F32, kind="ExternalInput")
    Ub_d = nc.dram_tensor("U_b", [S], F32, kind="ExternalInput")
    Ww_d = nc.dram_tensor("W_w", [S, S], F32, kind="ExternalInput")
    Wb_d = nc.dram_tensor("W_b", [S], F32, kind="ExternalInput")
    Vw_d = nc.dram_tensor("V_w", [S, M], F32, kind="ExternalInput")
    Vb_d = nc.dram_tensor("V_b", [M], F32, kind="ExternalInput")
    out_d = nc.dram_tensor("out", [BL, M], F32, kind="ExternalOutput")

    with tile.TileContext(nc) as tc:
        with (
            tc.tile_pool(name="singles", bufs=1) as singles,
            tc.tile_pool(name="ps_scan", bufs=6, space="PSUM") as ps_scan,
            tc.tile_pool(name="ps_xt", bufs=2, space="PSUM") as ps_xt,
        ):
            # ---- parameters ------------------------------------------------
            w_stage = singles.tile([S, S], F32, tag="wst")
            nc.sync.dma_start(out=w_stage, in_=Ww_d[:, :])
            w_sb = singles.tile([S, S], BF16, tag="w")         # W_w as lhsT
            nc.vector.tensor_copy(w_sb, w_stage)

            u_stage = singles.tile([N, S], F32, tag="ust")
            nc.sync.dma_start(out=u_stage, in_=Uw_d[:, :])
            u_sb = singles.tile([128, S], BF16, tag="u")       # U replicated x4
            for g in range(4):
                nc.vector.tensor_copy(u_sb[32 * g : 32 * g + N, :], u_stage)

            ub_sb = singles.tile([S, 1], F32, tag="ub")
            nc.sync.dma_start(out=ub_sb, in_=Ub_d[:].rearrange("(s o) -> s o", o=1))
            wb_sb = singles.tile([S, 1], F32, tag="wb")
            nc.sync.dma_start(out=wb_sb, in_=Wb_d[:].rearrange("(s o) -> s o", o=1))
            bias_sb = singles.tile([S, 1], F32, tag="bias")
            nc.vector.tensor_tensor(
                out=bias_sb, in0=ub_sb, in1=wb_sb, op=ALU.add
            )

            v_sb = singles.tile([S, M], F32, tag="v")
            nc.sync.dma_start(out=v_sb, in_=Vw_d[:, :])
            vb_row = singles.tile([1, M], F32, tag="vb")
            nc.sync.dma_start(out=vb_row, in_=Vb_d[:].rearrange("(o m) -> o m", o=1))
            ones_row = singles.tile([1, 128], F32, tag="ones")
            nc.vector.memset(ones_row, 1.0)

            ident = singles.tile([128, 128], F32, tag="ident")
            make_identity(nc, ident)

            # ---- scan state A^T [S, BL], ping-pong -------------------------
            # separate tile per (parity, half) so the two half-chains have no
            # false tile-granular dependencies between them
            a_t = [
                [
                    singles.tile([S, HALF], BF16, tag=f"a{i}_{h}", name=f"a{i}_{h}")
                    for h in range(2)
                ]
                for i in range(2)
            ]

            # load a0 -> A^T via 4 TensorE transposes
            for cb in range(NBC):
                a0_sb = singles.tile([128, S], F32, tag=f"a0in{cb}")
                nc.sync.dma_start(
                    out=a0_sb, in_=a0_d[cb * 128 : (cb + 1) * 128, :]
                )
                pt = ps_xt.tile([128, 512], F32, tag="xtp")
                nc.tensor.transpose(pt[:, 0:128], a0_sb, ident)
                h, hc = divmod(cb * 128, HALF)
                nc.scalar.copy(
                    out=a_t[0][h][:, hc : hc + 128], in_=pt[:, 0:128]
                )

            # ---- x streaming buffers --------------------------------------
            # xp[j][cb]: [128b, TBLK*NPAD] padded input block (j = block parity)
            # xt[j][q] : [128 (4l x 32n), BL] transposed block, q = quad in blk
            xp = [
                [singles.tile([128, TBLK * NPAD], F32, tag=f"xp{j}_{cb}",
                              name=f"xp{j}_{cb}")
                 for cb in range(NBC)]
                for j in range(2)
            ]
            xt = [
                [singles.tile([128, BL], BF16, tag=f"xt{j}_{q}",
                              name=f"xt{j}_{q}")
                 for q in range(TBLK // LQ)]
                for j in range(2)
            ]
            # zero-fill once so pad columns stay finite & initialized
            for j in range(2):
                for cb in range(NBC):
                    nc.gpsimd.memset(xp[j][cb], 0.0)

            # ---- main loop -------------------------------------------------
            for jb in range(NBLK):
                j = jb % 2
                # stream x block: [128, TBLK, 12] -> padded [128, TBLK, 32]
                for cb in range(NBC):
                    dst = xp[j][cb].rearrange("p (l n) -> p l n", n=NPAD)[:, :, 0:N]
                    nc.sync.dma_start(
                        out=dst,
                        in_=x_d[
                            cb * 128 : (cb + 1) * 128,
                            jb * TBLK : (jb + 1) * TBLK,
                            :,
                        ],
                    )
                # transpose quads: 4 b-chunks -> one [128, BL] xt tile
                for q in range(TBLK // LQ):
                    pt = ps_xt.tile([128, 512], F32, tag="xtp")
                    for cb in range(NBC):
                        nc.tensor.transpose(
                            pt[:, cb * 128 : (cb + 1) * 128],
                            xp[j][cb][:, q * 128 : (q + 1) * 128],
                            ident,
                        )
                    if q % 4 == 3:
                        nc.vector.tensor_copy(xt[j][q], pt)
                    else:
                        nc.scalar.copy(out=xt[j][q], in_=pt)

                # scan steps of this block
                for lt in range(TBLK):
                    l = jb * TBLK + lt
                    q, g = lt // LQ, lt % LQ
                    a_prev = a_t[l % 2]
                    a_new = a_t[(l + 1) % 2]
                    for h in range(2):
                        cols = slice(h * HALF, (h + 1) * HALF)
                        ps = ps_scan.tile([128, HALF], F32, tag="scan")
                        nc.tensor.matmul(
                            ps,
                            u_sb[32 * g : 32 * g + N, :],
                            xt[j][q][32 * g : 32 * g + N, cols],
                            start=True,
                            stop=False,
                            tile_position=(32 * g, 0),
                        )
                        nc.tensor.matmul(
                            ps, w_sb, a_prev[h], start=False, stop=True
                        )
                        if h == 0:
                            nc.scalar.activation(
                                a_new[h], ps, AF.Relu, bias=bias_sb, scale=1.0
                            )
                        else:
                            nc.vector.tensor_scalar(
                                out=a_new[h],
                                in0=ps,
                                scalar1=bias_sb,
                                scalar2=0.0,
                                op0=ALU.add,
                                op1=ALU.max,
                            )

            # ---- output: out[b, m] = A^T.T @ V_w + V_b ---------------------
            a_last = a_t[L % 2]
            af32 = [
                singles.tile([S, HALF], F32, tag=f"af32_{h}", name=f"af32_{h}")
                for h in range(2)
            ]
            for h in range(2):
                nc.vector.tensor_copy(af32[h], a_last[h])
            for cb in range(NBC):
                h, hc = divmod(cb * 128, HALF)
                po = ps_xt.tile([128, 512], F32, tag="xtp")
                nc.tensor.matmul(
                    po[:, 0:M], ones_row, vb_row, start=True, stop=False
                )
                nc.tensor.matmul(
                    po[:, 0:M],
                    af32[h][:, hc : hc + 128],
                    v_sb,
                    start=False,
                    stop=True,
                )
                o_sb = singles.tile([128, M], F32, tag=f"osb{cb}")
                nc.scalar.copy(out=o_sb, in_=po[:, 0:M])
                nc.sync.dma_start(
                    out=out_d[cb * 128 : (cb + 1) * 128, :], in_=o_sb
                )

    _split_multi_waits(nc)
    return nc


_CACHED_NC = None


def _get_nc():
    global _CACHED_NC
    if _CACHED_NC is None:
        _CACHED_NC = _build()
    return _CACHED_NC


def kernel(**inputs):
    x = np.ascontiguousarray(np.asarray(inputs["x"], dtype=np.float32))
    a0 = np.ascontiguousarray(np.asarray(inputs["a0"], dtype=np.float32))
    params = {
        k: np.ascontiguousarray(np.asarray(inputs[k], dtype=np.float32))
        for k in ("U_w", "U_b", "W_w", "W_b", "V_w", "V_b")
    }

    nc = _get_nc()
    in_maps = []
    for i in range(NCORES):
        m = {
            "x": x[i * BL : (i + 1) * BL],
            "a0": a0[i * BL : (i + 1) * BL],
        }
        m.update(params)
        in_maps.append(m)

    res = run_bass_kernel_spmd(nc, in_maps, core_ids=list(range(NCORES)))
    out = np.concatenate([res.results[i]["out"] for i in range(NCORES)], axis=0)
    return out.astype(np.float32)
